# revision 5
# baseline (speedup 1.0000x reference)
"""Trainium2 Bass kernel for nn_Conv2dLocal (locally-connected 2d conv,
no weight sharing).

Strategy: shard the 32 output rows across 8 NeuronCores (4 rows each).
Host pre-packs weights into fp8e3 (E3M4, x256 scale folded into x as /256)
and x into fp16 DMA-friendly layouts; per-location [K=576] contractions
run as fp8xfp16 matmuls with fp32 PSUM accumulation. Since walrus emits
no fast-weight-load, LDWEIGHTS columns are the bottleneck: K is chunked
into five K=128 chunks per location pair (kw{0,1} pairs via a +1-pixel
duplicated x tile; the kw=2 rows merged via a +1-row duplicated x tile)
plus two K=64 edges placed in disjoint PE-array quadrants so they execute
concurrently. PSUM drains alternate between VectorE and ScalarE with the
bias add fused; output is stored fp16.
"""

import numpy as np
import ml_dtypes

import concourse.bass as bass  # noqa: F401  (bass types referenced via bacc)
import concourse.mybir as mybir
import concourse.tile as tile
from concourse import bacc
from concourse.bass_utils import run_bass_kernel_spmd

# problem shape (hardcoded per contest contract)
B = 64
C = 64
H = W = 32
O = 64
OH = OW = 32
N_CORES = 8
R = 4  # oh rows per core
XW = 34  # padded width entries (-1..32)
XCOLS = 6 * XW * B  # 13056
WCOLS = 32 * 768   # wf: 24576
WMCOLS = 32 * 256  # whm: 8192
WECOLS = 32 * 128  # whe: 4096
OWB = 8  # ow columns per weight DMA block
WSCALE = 256.0  # fp8 weight scale; x is pre-divided by this on host
F8 = mybir.dt.float8e3
F16 = mybir.dt.float16
F32 = mybir.dt.float32
NP_F8 = ml_dtypes.float8_e3m4
IDENT = mybir.ActivationFunctionType.Identity

_NC_CACHE = {}


def build_kernel(n_iter=1, n_cores=N_CORES, w_bufs=3, psum_bufs=8):
    nc = bacc.Bacc("TRN2", target_bir_lowering=False, debug=False,
                   num_devices=n_cores)

    x_d = nc.dram_tensor("xp", [64, XCOLS], F16, kind="ExternalInput")
    wf_d = nc.dram_tensor("wf", [128, WCOLS], F8, kind="ExternalInput")
    wm_d = nc.dram_tensor("wm", [128, WMCOLS], F8, kind="ExternalInput")
    we_d = nc.dram_tensor("we", [128, WECOLS], F8, kind="ExternalInput")
    b_d = nc.dram_tensor("bias", [128, 64], F32, kind="ExternalInput")
    o_d = nc.dram_tensor("out", [2, 128, 2048], F16, kind="ExternalOutput")

    with tile.TileContext(nc) as tc:
        with (
            tc.tile_pool(name="xpool", bufs=1) as xpool,
            tc.tile_pool(name="cpool", bufs=1) as cpool,
            tc.tile_pool(name="opool", bufs=1) as opool,
            tc.tile_pool(name="wfpool", bufs=w_bufs) as wfpool,
            tc.tile_pool(name="wmpool", bufs=w_bufs) as wmpool,
            tc.tile_pool(name="wepool", bufs=w_bufs) as wepool,
            tc.tile_pool(name="pspool", bufs=psum_bufs, space="PSUM") as pspool,
        ):
            # x_sb: partitions 0-63 direct; 64-127 shifted one PIXEL (+64)
            x_sb = xpool.tile([128, XCOLS], F16, name="x_sb")
            nc.sync.dma_start(out=x_sb[0:64, :], in_=x_d[:, :])
            nc.sync.dma_start(out=x_sb[64:128, 0 : XCOLS - 64],
                              in_=x_d[:, 64:XCOLS])
            # x2_sb: partitions 0-63 direct; 64-127 shifted one ROW (+34px)
            x2_sb = xpool.tile([128, XCOLS], F16, name="x2_sb")
            nc.sync.dma_start(out=x2_sb[0:64, :], in_=x_d[:, :])
            nc.sync.dma_start(out=x2_sb[64:128, 0 : XCOLS - 34 * B],
                              in_=x_d[:, 34 * B : XCOLS])

            bias_sb = cpool.tile([128, 64], F32)
            nc.sync.dma_start(out=bias_sb[:], in_=b_d[:, :])

            out_sb = [
                opool.tile([128, 2048], F16, tag=f"out{p}", name=f"out_sb{p}")
                for p in (0, 1)
            ]

            def body():
                for blk in range(OW // OWB):
                    wf_t = wfpool.tile([128, OWB * 768], F8, name="wf_t")
                    wm_t = wmpool.tile([128, OWB * 256], F8, name="wm_t")
                    we_t = wepool.tile([128, OWB * 128], F8, name="we_t")
                    nc.sync.dma_start(
                        out=wf_t[:],
                        in_=wf_d[:, blk * OWB * 768 : (blk + 1) * OWB * 768])
                    nc.sync.dma_start(
                        out=wm_t[:],
                        in_=wm_d[:, blk * OWB * 256 : (blk + 1) * OWB * 256])
                    nc.sync.dma_start(
                        out=we_t[:],
                        in_=we_d[:, blk * OWB * 128 : (blk + 1) * OWB * 128])
                    for j in range(OWB):
                        ow = blk * OWB + j
                        for p in (0, 1):
                            ps = pspool.tile([128, 64], F32, name="ps")
                            bf = j * 768 + p * 384
                            bm = j * 256 + p * 128
                            be = j * 128 + p * 64
                            hA = 1 + 2 * p
                            cF = lambda h: (h * XW + ow) * B
                            cH = lambda h: (h * XW + ow + 2) * B
                            mm = nc.tensor.matmul
                            # kw{0,1} K=128 chunks; M=128 packs the two
                            # oh-adjacent locs sharing the rhs pixel pair
                            mm(ps[0:128, :], wf_t[:, bf : bf + 128],
                               x_sb[:, cF(hA) : cF(hA) + 64],
                               start=True, stop=False)
                            mm(ps[0:128, :], wf_t[:, bf + 128 : bf + 256],
                               x_sb[:, cF(hA + 1) : cF(hA + 1) + 64],
                               start=False, stop=False)
                            mm(ps[0:64, :], wf_t[:, bf + 256 : bf + 320],
                               x_sb[:, cF(hA - 1) : cF(hA - 1) + 64],
                               start=False, stop=False)
                            mm(ps[64:128, :], wf_t[:, bf + 320 : bf + 384],
                               x_sb[:, cF(hA + 2) : cF(hA + 2) + 64],
                               start=False, stop=False)
                            # kw=2 rows hA/hA+1 merged via +1-row x copy
                            mm(ps[0:128, :], wm_t[:, bm : bm + 128],
                               x2_sb[:, cH(hA) : cH(hA) + 64],
                               start=False, stop=False)
                            # kw=2 edge rows: disjoint quadrants (0,0)/(64,64)
                            mm(ps[0:64, :], we_t[0:64, be : be + 64],
                               x_sb[0:64, cH(hA - 1) : cH(hA - 1) + 64],
                               start=False, stop=False)
                            mm(ps[64:128, :], we_t[64:128, be : be + 64],
                               x_sb[64:128, cH(hA + 2) - 64 : cH(hA + 2)],
                               start=False, stop=True)
                            jcol = p * 32 + ow
                            if p == 0:
                                nc.vector.tensor_scalar_add(
                                    out=out_sb[p][:, ow * 64 : (ow + 1) * 64],
                                    in0=ps[:, :],
                                    scalar1=bias_sb[:, jcol : jcol + 1],
                                )
                            else:
                                nc.scalar.activation(
                                    out=out_sb[p][:, ow * 64 : (ow + 1) * 64],
                                    in_=ps[:, :],
                                    func=IDENT,
                                    bias=bias_sb[:, jcol : jcol + 1],
                                    scale=1.0,
                                )

                for p in (0, 1):
                    nc.sync.dma_start(out=o_d[p], in_=out_sb[p][:])

            if n_iter == 1:
                body()
            else:
                with tc.For_i(0, n_iter, 1):
                    body()

    nc.compile()
    return nc


def get_nc():
    if "nc" not in _NC_CACHE:
        _NC_CACHE["nc"] = build_kernel()
    return _NC_CACHE["nc"]


# ---------------- host-side layout prep ----------------

def prep_x(x):
    xt = x.transpose(1, 2, 3, 0)  # [c, h, w, b]
    xp = np.zeros((C, H + 2, W + 2, B), np.float16)
    xp[:, 1 : H + 1, 1 : W + 1, :] = xt * np.float32(1.0 / WSCALE)
    return [
        np.ascontiguousarray(xp[:, R * c : R * c + 6, :, :].reshape(C, XCOLS))
        for c in range(N_CORES)
    ]


def prep_w(weight):
    w8 = (weight * np.float32(WSCALE)).astype(NP_F8)
    wfs, wms, wes = [], [], []
    for core in range(N_CORES):
        r0 = R * core
        Wc = w8[r0 : r0 + 4]  # [4, 32, O, C, KH, KW]
        # TF[ohl, ow, kh] = [(kw01, c) = 128 rows, o = 64 cols]
        TF = (Wc[:, :, :, :, :, 0:2]
              .transpose(0, 1, 4, 5, 3, 2).reshape(4, 32, 3, 128, O))
        # TH[ohl, ow, kh] = [c = 64 rows, o = 64 cols]  (kw=2)
        TH = Wc[:, :, :, :, :, 2].transpose(0, 1, 4, 3, 2)
        fulls, mids, edges = [], [], []
        for p in (0, 1):
            A, Bb = 2 * p, 2 * p + 1
            FP1 = np.concatenate([TF[A, :, 1], TF[Bb, :, 0]], axis=-1)
            FP2 = np.concatenate([TF[A, :, 2], TF[Bb, :, 1]], axis=-1)
            fulls.append(np.concatenate(
                [FP1, FP2, TF[A, :, 0], TF[Bb, :, 2]], axis=-1))
            # wm: rows 0-63 = (A kh1 | B kh0) kw2, rows 64-127 = (A kh2 | B kh1)
            HP1 = np.concatenate([TH[A, :, 1], TH[Bb, :, 0]], axis=-1)
            HP2 = np.concatenate([TH[A, :, 2], TH[Bb, :, 1]], axis=-1)
            mids.append(np.concatenate([HP1, HP2], axis=1))  # [32,128,128]
            # we: rows 0-63 = A kh0 kw2, rows 64-127 = B kh2 kw2
            edges.append(np.concatenate([TH[A, :, 0], TH[Bb, :, 2]], axis=1))
        wfull = np.concatenate(fulls, axis=-1)   # [32, 128, 768]
        wmid = np.concatenate(mids, axis=-1)     # [32, 128, 256]
        wedge = np.concatenate(edges, axis=-1)   # [32, 128, 128]
        wfs.append(np.ascontiguousarray(
            wfull.transpose(1, 0, 2).reshape(128, WCOLS)))
        wms.append(np.ascontiguousarray(
            wmid.transpose(1, 0, 2).reshape(128, WMCOLS)))
        wes.append(np.ascontiguousarray(
            wedge.transpose(1, 0, 2).reshape(128, WECOLS)))
    return wfs, wms, wes


def prep_bias(bias):
    outs = []
    for core in range(N_CORES):
        bc = bias[:, R * core : R * core + 4, :]  # [O, 4, OW]
        b0 = np.concatenate([bc[:, 0], bc[:, 1]], axis=0)
        b1 = np.concatenate([bc[:, 2], bc[:, 3]], axis=0)
        outs.append(np.ascontiguousarray(
            np.concatenate([b0, b1], axis=1)).astype(np.float32))
    return outs


def make_in_maps(x, weight, bias):
    xs = prep_x(np.asarray(x, dtype=np.float32))
    wfs, wms, wes = prep_w(np.asarray(weight, dtype=np.float32))
    bs = prep_bias(np.asarray(bias, dtype=np.float32))
    return [
        {"xp": xs[c], "wf": wfs[c], "wm": wms[c], "we": wes[c], "bias": bs[c]}
        for c in range(N_CORES)
    ]


def assemble_out(per_core):
    out = np.empty((B, O, OH, OW), np.float32)
    for core in range(N_CORES):
        r0 = R * core
        dev = np.asarray(per_core[core], np.float32).reshape(2, 2, O, OW, B)
        for p in (0, 1):
            for half in (0, 1):
                out[:, :, r0 + 2 * p + half, :] = dev[p, half].transpose(2, 0, 1)
    return out


def kernel(x, weight, bias):
    nc = get_nc()
    in_maps = make_in_maps(x, weight, bias)
    res = run_bass_kernel_spmd(nc, in_maps, core_ids=list(range(N_CORES)))
    return assemble_out([res.results[c]["out"] for c in range(N_CORES)])
